# revision 1
# baseline (speedup 1.0000x reference)
"""Trainium2 Bass kernel for nn_CGATLayer (GNN message passing).

Algorithm (matches reference):
    z = feature @ fc_weight                      # [N, D]
    s = z @ attn[:D];  d = z @ attn[D:]          # per-node scalars
    e[n,j]   = leaky_relu(s[src[n,j]] + d[n])
    alpha[n,j] = sum_k relu(e[n,j] - e[n,k])
    h[n]     = sum_j alpha[n,j] * z[src[n,j]]

Device strategy (8 NeuronCores, SPMD single program):
  - dest nodes sharded 6272/core (49 full 128-row tiles; global padding rows
    are computed then discarded on the host); weights replicated.
  - phase A (replicated): every core builds a DRAM table [N, 128] u16 with
    256 B row stride but only 132 B payload per row:
      u16 cols 0:64 = z row (bf16), f32 col 32 = s2 = 0.5*s.
    Using e' = e/2 (positive homogeneity of leaky_relu/relu):
      alpha = sum_k |e'_j - e'_k| + DEG*e'_j - sum_k e'_k
    so the pairwise clamp reduction is one abs-reduce.  feature ships from
    the host pre-transposed in bf16, halving the input read, and matmuls run
    in bf16.
  - the dest-side bias d2 = 0.5*d is NOT gathered: each core receives its own
    feature slice (featOwnT) and computes d2 for its 49 dest tiles with one
    tiny PE matmul per tile (PE is otherwise idle), keeping the program SPMD.
  - phase B: per 128-dest tile ONE dma_gather fetches 4096 512-byte two-row
    blocks at block index src>>1 (gather indices are int16, so row indices up
    to 50000 are addressed as 25000 blocks).  The Pool queue carries ONLY the
    gathers so descriptor generation for tile t+1 overlaps tile t's transfer.
    The row parity (src & 1) is resolved IN PLACE with copy_predicated (int16
    mask; for z on the f32-bitcast view, which halves the element count), then
      e' = Lrelu(s2_sel + d2) runs on the Activation engine (per-partition
      bias + free row-sum accumulator), the pairwise diff / abs-reduce / alpha
      on DVE, alpha is broadcast to [P,deg,64] bf16 on Act so the weighted-sum
      multiply runs in packed 2x DVE mode, and a fold + strided reduce give h.
    DMA queue assignment avoids head-of-line blocking: index loads + h writes
    on SP, table writes on Act, gathers on Pool.
"""

from contextlib import ExitStack

import numpy as np

import concourse.bass as bass
import concourse.bacc as bacc
import concourse.tile as tile
from concourse import mybir

F32 = mybir.dt.float32
BF16 = mybir.dt.bfloat16
I16 = mybir.dt.int16
U16 = mybir.dt.uint16
ALU = mybir.AluOpType
AXL = mybir.AxisListType
ACT = mybir.ActivationFunctionType

N, DEG, IN_DIM, OUT_DIM = 50000, 32, 128, 64
NCORES = 8
NEG_SLOPE = 0.01
P = 128
PN = 6272                    # dest rows per core (49 tiles of 128)
NTILES = PN // P


def build_program(n=N, pn=PN, deg=DEG, in_dim=IN_DIM, out_dim=OUT_DIM, cg=7,
                  ncores=NCORES, xs=None):
    """Build the SPMD Bass program. pn = dest nodes owned by this core."""
    ntiles = pn // P
    row = 128                    # u16 units per table row slot (256 B stride)
    rowp = out_dim + 2           # u16 units actually written (z bf16 + s2 f32)
    scol = out_dim // 2          # f32 col of s2 within a row
    nidx = deg * P               # gathered blocks per tile
    wcols = nidx // 16           # wrapped-index columns
    nchunks = (n + P - 1) // P
    ngroups = (nchunks + cg - 1) // cg
    nblk = (n + 1) // 2
    if xs is None:
        xs = [nblk] * (pn // P)

    nc = bacc.Bacc("TRN2", num_devices=ncores,
                   dynamic_dma_scratch_size=16384, num_swdge_queues=2)
    featT = nc.declare_dram_parameter("featT", [in_dim, n], BF16, isOutput=False)
    featOwnT = nc.declare_dram_parameter("featOwnT", [in_dim, pn], BF16,
                                         isOutput=False)
    fc = nc.declare_dram_parameter("fc", [in_dim, out_dim], BF16, isOutput=False)
    fcT = nc.declare_dram_parameter("fcT", [out_dim, in_dim], BF16,
                                    isOutput=False)
    attn2 = nc.declare_dram_parameter("attn2", [out_dim, 2], BF16,
                                      isOutput=False)
    idxw = nc.declare_dram_parameter("idxw", [ntiles * P, wcols + deg], I16,
                                     isOutput=False)
    h = nc.declare_dram_parameter("h", [pn, out_dim], F32, isOutput=True)
    table = nc.dram_tensor("table", [n, row], U16)

    with tile.TileContext(nc) as tc, ExitStack() as ctx:
        const_pool = ctx.enter_context(tc.tile_pool(name="const", bufs=1))
        ft_pool = ctx.enter_context(tc.tile_pool(name="ft", bufs=4))
        row_pool = ctx.enter_context(tc.tile_pool(name="rowp", bufs=4))
        psA_pool = ctx.enter_context(tc.tile_pool(name="psA", bufs=4,
                                                  space="PSUM"))
        psS_pool = ctx.enter_context(tc.tile_pool(name="psS", bufs=1,
                                                  space="PSUM"))
        g_pool = ctx.enter_context(tc.tile_pool(name="g", bufs=5))
        it_pool = ctx.enter_context(tc.tile_pool(name="it", bufs=6))
        sm_pool = ctx.enter_context(tc.tile_pool(name="sm", bufs=6))
        D_pool = ctx.enter_context(tc.tile_pool(name="Dp", bufs=4))
        pr_pool = ctx.enter_context(tc.tile_pool(name="pr", bufs=4))
        h_pool = ctx.enter_context(tc.tile_pool(name="hp", bufs=4))

        # ---- weight prep: R = [fc | 0.5*fc@a1 | 0.5*fc@a2]  [in_dim, 66] bf16
        # (attn2 is pre-scaled by +0.5 on the host for both columns)
        fc_sb = const_pool.tile([in_dim, out_dim], BF16)
        nc.sync.dma_start(fc_sb[:], fc[:])
        fcT_sb = const_pool.tile([out_dim, in_dim], BF16)
        nc.sync.dma_start(fcT_sb[:], fcT[:])
        attn2_sb = const_pool.tile([out_dim, 2], BF16)
        nc.sync.dma_start(attn2_sb[:], attn2[:])
        R_sb = const_pool.tile([in_dim, out_dim + 2], BF16)
        wsd_ps = psS_pool.tile([in_dim, 2], F32, tag="psS")
        nc.tensor.matmul(out=wsd_ps[:], lhsT=fcT_sb[:], rhs=attn2_sb[:],
                         start=True, stop=True)
        nc.vector.tensor_copy(out=R_sb[:, 0:out_dim], in_=fc_sb[:])
        nc.vector.tensor_copy(out=R_sb[:, out_dim:out_dim + 2], in_=wsd_ps[:])

        # ---- d2 for own dest rows: one [128,128]x[128,1] matmul per tile
        fo = const_pool.tile([in_dim, pn], BF16)
        nc.sync.dma_start(fo[:], featOwnT[:])
        d2_ps = psS_pool.tile([P, ntiles], F32, tag="psS2")
        for t in range(ntiles):
            nc.tensor.matmul(out=d2_ps[:, t:t + 1],
                             lhsT=fo[:, t * P:(t + 1) * P],
                             rhs=R_sb[:, out_dim + 1:out_dim + 2],
                             start=True, stop=True)
        d2 = const_pool.tile([P, ntiles], F32)
        nc.vector.tensor_copy(out=d2[:], in_=d2_ps[:])

        # ---- phase A: build table (replicated: every core computes all rows)
        ocols = out_dim + 2
        for gi in range(ngroups):
            c0 = gi * cg
            cn = min(cg, nchunks - c0)
            n0 = c0 * P
            nn = min(n - n0, cn * P)
            ft = ft_pool.tile([P, cg * P], BF16, tag="ft")
            nc.sync.dma_start(ft[:, :nn], featT[:, n0:n0 + nn])
            ps = psA_pool.tile([P, cg * ocols], F32, tag="psA")
            for q in range(cn):
                cw = min(P, n - (c0 + q) * P)
                nc.tensor.matmul(out=ps[:cw, q * ocols:(q + 1) * ocols],
                                 lhsT=ft[:, q * P:q * P + cw], rhs=R_sb[:],
                                 start=True, stop=True)
            rowt = row_pool.tile([P, cg * rowp], U16, tag="rowt")
            ps3 = ps[:].rearrange("p (q f) -> p q f", f=ocols)
            rowb3 = rowt[:].bitcast(BF16).rearrange("p (q f) -> p q f", f=rowp)
            rowf3 = rowt[:].bitcast(F32).rearrange("p (q f) -> p q f",
                                                   f=rowp // 2)
            nc.vector.tensor_copy(out=rowb3[:, 0:cn, 0:out_dim],
                                  in_=ps3[:, 0:cn, 0:out_dim])
            nc.vector.tensor_copy(out=rowf3[:, 0:cn, scol:scol + 1],
                                  in_=ps3[:, 0:cn, out_dim:out_dim + 1])
            if nn == cn * P:
                tv = table[n0:n0 + nn, 0:rowp].rearrange("(q p) f -> p q f",
                                                         p=P)
                nc.scalar.dma_start(
                    out=tv,
                    in_=rowt[:].rearrange("p (q f) -> p q f", f=rowp)[:, 0:cn, :])
            else:
                for q in range(cn):
                    cw = min(P, n - (c0 + q) * P)
                    nc.scalar.dma_start(
                        out=table[(c0 + q) * P:(c0 + q) * P + cw, 0:rowp],
                        in_=rowt[:cw, q * rowp:(q + 1) * rowp])

        # ---- phase B: per-dest-tile block gather + attention + weighted sum
        # Software-pipelined one tile deep: iteration t issues the gather for
        # tile t, then the math for tile t-1.  The pairwise diff D runs on
        # Pool; since its input e(t-1) is ready before gather t's transfer
        # completes, it never delays desc-gen for gather t+1 on the Pool queue.

        def stage1(t, g, mks):
            """s-select + e' + pairwise diff (inputs: gather t just landed)."""
            gf3 = g[:].bitcast(F32).rearrange("p (j f) -> p j f", f=row)
            slo = gf3[:, :, scol:scol + 1].rearrange("p j one -> p (j one)")
            shi = gf3[:, :, (row // 2) + scol:(row // 2) + scol + 1].rearrange(
                "p j one -> p (j one)")
            # parity-resolve s in place: slo <- shi where parity=1
            nc.vector.copy_predicated(out=slo, mask=mks, data=shi)
            # e' = Lrelu(s2_sel + d2_dest); sumE = sum_j e'  (free accumulator)
            e = sm_pool.tile([P, deg], F32, tag="e")
            sumE = sm_pool.tile([P, 1], F32, tag="sumE")
            nc.scalar.activation(out=e[:], in_=slo, func=ACT.Lrelu,
                                 bias=d2[:, t:t + 1], scale=1.0,
                                 alpha=NEG_SLOPE, accum_out=sumE[:])
            # D[p,j,k] = e'_j - e'_k  (broadcast diff is cheap on DVE)
            D = D_pool.tile([P, deg * deg], F32, tag="D")
            D3 = D[:].rearrange("p (j k) -> p j k", k=deg)
            nc.vector.tensor_tensor(
                out=D3, in0=e[:].unsqueeze(2).broadcast_to([P, deg, deg]),
                in1=e[:].unsqueeze(1).broadcast_to([P, deg, deg]),
                op=ALU.subtract)
            return e, sumE, D3

        def stage2(t, g, mks, e, sumE, D3):
            """alpha + weighted sum (inputs one full period old)."""
            gf3 = g[:].bitcast(F32).rearrange("p (j f) -> p j f", f=row)
            gb3 = g[:].bitcast(BF16).rearrange("p (j f) -> p j f", f=2 * row)
            A = sm_pool.tile([P, deg], F32, tag="A")
            nc.vector.tensor_reduce(out=A[:], in_=D3, axis=AXL.X, op=ALU.add,
                                    apply_absolute_value=True)
            # alpha = A + deg*e' - sumE   (bf16 for the weighted sum)
            al0 = sm_pool.tile([P, deg], F32, tag="al0")
            nc.vector.tensor_scalar(out=al0[:], in0=e[:], scalar1=float(deg),
                                    scalar2=sumE[:], op0=ALU.mult,
                                    op1=ALU.subtract)
            alpha = sm_pool.tile([P, deg], BF16, tag="alpha")
            nc.vector.tensor_tensor(out=alpha[:], in0=al0[:], in1=A[:],
                                    op=ALU.add)
            # broadcast alpha to [P, deg, out_dim] bf16 on Act so the multiply
            # below runs fully packed (2x DVE mode)
            al64 = pr_pool.tile([P, deg * out_dim], BF16, tag="al64")
            al64v = al64[:].rearrange("p (j f) -> p j f", f=out_dim)
            nc.scalar.activation(
                out=al64v,
                in_=alpha[:].unsqueeze(2).broadcast_to([P, deg, out_dim]),
                func=ACT.Copy)
            # parity-resolve z in place on the f32-bitcast view (half the
            # elements; a bf16 pair shares its row's parity): zlo <- zhi
            # (overlaps the Act broadcast above on DVE)
            zlo = gb3[:, :, 0:out_dim]
            zlo32 = gf3[:, :, 0:out_dim // 2]
            zhi32 = gf3[:, :, row // 2:row // 2 + out_dim // 2]
            nc.vector.copy_predicated(
                out=zlo32,
                mask=mks.unsqueeze(2).broadcast_to([P, deg, out_dim // 2]),
                data=zhi32)
            # h = sum_j alpha_j * z_j  (fold j 32->16, then strided reduce)
            prod = pr_pool.tile([P, deg * out_dim], BF16, tag="prod")
            prod3 = prod[:].rearrange("p (j d) -> p j d", d=out_dim)
            nc.vector.tensor_tensor(out=prod3, in0=zlo, in1=al64v, op=ALU.mult)
            half = pr_pool.tile([P, (deg // 2) * out_dim], BF16, tag="half")
            half3 = half[:].rearrange("p (j d) -> p j d", d=out_dim)
            nc.vector.tensor_tensor(out=half3, in0=prod3[:, 0:deg // 2, :],
                                    in1=prod3[:, deg // 2:deg, :], op=ALU.add)
            hsb = h_pool.tile([P, out_dim], F32, tag="hsb")
            pv = half3.transpose([0, 2, 1])
            nc.vector.tensor_reduce(out=hsb[:], in_=pv, axis=AXL.X, op=ALU.add)
            nc.sync.dma_start(out=h[t * P:t * P + P, :], in_=hsb[:])

        nidx_reg = nc.gpsimd.to_reg(nidx)
        s0 = s1 = None
        for t in range(ntiles + 2):
            if t < ntiles:
                it2 = it_pool.tile([P, wcols + deg], I16, tag="it2")
                nc.sync.dma_start(it2[:], idxw[t * P:(t + 1) * P, :])
                g = g_pool.tile([P, nidx * 2], U16, tag="g")  # 32 blk x 256 u16
                g3v = g[:].rearrange("p (j f) -> p j f", f=2 * row)
                # source view sliced to this tile's max block prefix: the
                # gather then only depends on the table rows written so far,
                # letting early (sorted) tiles start during phase A
                tblv = table[0:2 * xs[t], :].rearrange(
                    "(b two) f -> b (two f)", two=2)
                nc.gpsimd.dma_gather(out_ap=g3v, in_ap=tblv,
                                     idxs_ap=it2[:, 0:wcols],
                                     num_idxs=nidx, num_idxs_reg=nidx_reg,
                                     elem_size=2 * row, single_packet=False,
                                     queue_num=t % 2)
            if s0 is not None:
                mid = stage1(s0[0], s0[1], s0[2])
            if s1 is not None:
                stage2(s1[0], s1[1], s1[2], *s1[3])
            s1 = (s0[0], s0[1], s0[2], mid) if s0 is not None else None
            s0 = (t, g, it2[:, wcols:wcols + deg]) if t < ntiles else None

    nc.compile()
    return nc


def prep_inputs(feature, src_idx, fc_weight, attn_weight, ncores=NCORES):
    """Host-side sharding/layout prep -> per-core input maps."""
    import ml_dtypes

    bf16 = ml_dtypes.bfloat16
    feature = np.asarray(feature, dtype=np.float32)
    src = np.asarray(src_idx).astype(np.int64)
    fcw = np.asarray(fc_weight, dtype=np.float32)
    aw = np.asarray(attn_weight, dtype=np.float32)
    n, in_dim = feature.shape
    out_dim = fcw.shape[1]
    deg = src.shape[1]
    pn = PN
    ntiles = pn // P
    wcols = deg * P // 16

    featT = np.ascontiguousarray(feature.T).astype(bf16)
    fcb = fcw.astype(bf16)
    fcT = np.ascontiguousarray(fcw.T).astype(bf16)
    attn2 = np.ascontiguousarray(
        np.stack([0.5 * aw[:out_dim, 0], 0.5 * aw[out_dim:, 0]],
                 axis=1)).astype(bf16)

    # padded per-core dest rows (cores own [c*pn, (c+1)*pn); rows >= n are
    # dummies whose outputs are discarded)
    src_pad = np.zeros((ncores * pn, deg), dtype=np.int64)
    src_pad[:n] = src
    featT_pad = np.zeros((in_dim, ncores * pn), dtype=bf16)
    featT_pad[:, :n] = featT

    in_maps = []
    orders = []
    tile_max = np.zeros((ncores, ntiles), dtype=np.int64)
    for c in range(ncores):
        cols = src_pad[c * pn:(c + 1) * pn]
        # sort dests by max source block so early tiles only need a prefix
        # of the z/s table (pad rows sort last)
        key = (cols >> 1).max(axis=1)
        key[c * pn + np.arange(pn) >= n] = 1 << 30
        order = np.argsort(key, kind="stable")
        orders.append(order)
        cols = cols[order]
        blk = (cols >> 1).astype(np.int16)           # two-row block index
        msk = (cols & 1).astype(np.int16)            # row parity within block
        for t in range(ntiles):
            tile_max[c, t] = int(blk[t * P:(t + 1) * P].max()) + 1
        idxw = np.zeros((ntiles * P, wcols + deg), dtype=np.int16)
        for t in range(ntiles):
            flat = blk[t * P:(t + 1) * P].T.reshape(-1)   # i = q*128 + p
            wrapped = flat.reshape(wcols, 16).T            # [16, wcols]
            idxw[t * P:(t + 1) * P, :wcols] = np.tile(wrapped, (8, 1))
        idxw[:, wcols:] = msk
        featOwn = featT_pad[:, c * pn:(c + 1) * pn][:, order]
        in_maps.append({"featT": featT,
                        "featOwnT": np.ascontiguousarray(featOwn),
                        "fc": fcb, "fcT": fcT, "attn2": attn2,
                        "idxw": idxw})
    xs = [int(tile_max[:, t].max()) for t in range(ntiles)]
    return in_maps, pn, orders, xs


_prog_cache = {}


def kernel(feature, src_idx, fc_weight, attn_weight):
    from concourse.bass_utils import run_bass_kernel_spmd

    in_maps, pn, orders, xs = prep_inputs(feature, src_idx, fc_weight,
                                          attn_weight)
    key = ("v5", feature.shape, pn, tuple(xs))
    if key not in _prog_cache:
        _prog_cache[key] = build_program(n=feature.shape[0], pn=pn, xs=xs)
    nc = _prog_cache[key]
    res = run_bass_kernel_spmd(nc, in_maps, list(range(NCORES)))
    n = feature.shape[0]
    h = np.zeros((NCORES * pn, OUT_DIM), dtype=np.float32)
    for c in range(NCORES):
        hc = np.asarray(res.results[c]["h"]).astype(np.float32)
        h[c * pn + orders[c]] = hc
    return np.ascontiguousarray(h[:n], dtype=np.float32)



# revision 5
# speedup vs baseline: 1.6042x; 1.6042x over previous
"""Trainium2 Bass kernel for nn_CGATLayer (GNN message passing).

Algorithm (matches reference):
    z = feature @ fc_weight                      # [N, D]
    s = z @ attn[:D];  d = z @ attn[D:]          # per-node scalars
    e[n,j]   = leaky_relu(s[src[n,j]] + d[n])
    alpha[n,j] = sum_k relu(e[n,j] - e[n,k])
    h[n]     = sum_j alpha[n,j] * z[src[n,j]]

Device strategy (8 NeuronCores, SPMD single program):
  im2col formulation: instead of computing z per node and gathering rows
  per edge (DMA-descriptor bound: one descriptor per edge), the host ships
  the feature matrix with columns replicated in per-edge order
  (featdup[:, (t, j, p)] = feature[src[t*128+p, j]].T, j=32 = the dest's
  own feature column).  Each dest tile then needs 33 PE matmuls
  [128x128]x[128x66] against R = [fc | 0.5*fc@a1 | 0.5*fc@a2] to produce
  per-edge [z | s2 | d2] directly in PSUM -- per-edge data delivery rides
  on contiguous full-bandwidth DMA + the idle PE array instead of 200k
  512-byte gather descriptors.

  Using e' = e/2 (positive homogeneity of leaky_relu/relu):
      alpha = sum_k |e'_j - e'_k| + DEG*e'_j - sum_k e'_k
  so the pairwise clamp reduction is one abs-reduce.

  Per tile: 33 matmuls (5 PSUM banks) -> Act evacuates each bank
  transposed/bf16 into zsT [128, 66, 33] (d-major so the j-reduction is
  innermost) -> Act e' = Lrelu(s2 + d2) with free row-sum accumulator ->
  DVE pairwise diff (bf16 out) -> Pool abs-reduce -> DVE alpha, alpha
  broadcast to [128, 64, 32] (4x tensor_copy), product, one fold ->
  Pool reduces [128, 64, 16] -> h slot; h written to DRAM every 8 tiles
  in a partition-major layout so each partition's rows are contiguous.
"""

from contextlib import ExitStack

import numpy as np

import concourse.bass as bass
import concourse.bacc as bacc
import concourse.tile as tile
from concourse import mybir

F32 = mybir.dt.float32
BF16 = mybir.dt.bfloat16
ALU = mybir.AluOpType
AXL = mybir.AxisListType
ACT = mybir.ActivationFunctionType

N, DEG, IN_DIM, OUT_DIM = 50000, 32, 128, 64
NCORES = 8
NEG_SLOPE = 0.01
P = 128
PN = 6272                    # dest rows per core (49 tiles of 128)
NTILES = PN // P
JB = DEG + 1                 # 32 edge blocks + 1 own-feature block (d2)
OC = OUT_DIM + 2             # matmul output cols: z(64) | s2 | d2
HGRP = 8                     # tiles batched per h write


def build_program(pn=PN, in_dim=IN_DIM, out_dim=OUT_DIM, ncores=NCORES):
    ntiles = pn // P
    bank_js = [7, 7, 7, 7, 5]            # j-blocks per PSUM bank (sum=33)
    nh = (ntiles + HGRP - 1) // HGRP

    nc = bacc.Bacc("TRN2", num_devices=ncores)
    fd = nc.declare_dram_parameter("fd", [in_dim, ntiles * JB * P], BF16,
                                   isOutput=False)
    fc = nc.declare_dram_parameter("fc", [in_dim, out_dim], BF16, isOutput=False)
    fcT = nc.declare_dram_parameter("fcT", [out_dim, in_dim], BF16,
                                    isOutput=False)
    attn2 = nc.declare_dram_parameter("attn2", [out_dim, 2], BF16,
                                      isOutput=False)
    h = nc.declare_dram_parameter("h", [P, ntiles * out_dim], F32,
                                  isOutput=True)

    with tile.TileContext(nc) as tc, ExitStack() as ctx:
        const_pool = ctx.enter_context(tc.tile_pool(name="const", bufs=1))
        fd_pool = ctx.enter_context(tc.tile_pool(name="fd", bufs=3))
        ps_pool = ctx.enter_context(tc.tile_pool(name="ps", bufs=1,
                                                 space="PSUM"))
        zs_pool = ctx.enter_context(tc.tile_pool(name="zs", bufs=3))
        sm_pool = ctx.enter_context(tc.tile_pool(name="sm", bufs=3))
        D_pool = ctx.enter_context(tc.tile_pool(name="Dp", bufs=3))
        pr_pool = ctx.enter_context(tc.tile_pool(name="pr", bufs=3))
        h_pool = ctx.enter_context(tc.tile_pool(name="hp", bufs=2))

        # ---- weight prep: R = [fc | 0.5*fc@a1 | 0.5*fc@a2]  [in_dim, 66] bf16
        # (attn2 is pre-scaled by 0.5 on the host for both columns)
        fc_sb = const_pool.tile([in_dim, out_dim], BF16)
        nc.sync.dma_start(fc_sb[:], fc[:])
        fcT_sb = const_pool.tile([out_dim, in_dim], BF16)
        nc.sync.dma_start(fcT_sb[:], fcT[:])
        attn2_sb = const_pool.tile([out_dim, 2], BF16)
        nc.sync.dma_start(attn2_sb[:], attn2[:])
        R_sb = const_pool.tile([in_dim, OC], BF16)
        wsd_ps = ps_pool.tile([in_dim, 2], F32, tag="psW")
        nc.tensor.matmul(out=wsd_ps[:], lhsT=fcT_sb[:], rhs=attn2_sb[:],
                         start=True, stop=True)
        nc.vector.tensor_copy(out=R_sb[:, 0:out_dim], in_=fc_sb[:])
        nc.vector.tensor_copy(out=R_sb[:, out_dim:OC], in_=wsd_ps[:])

        hbuf = None
        for t in range(ntiles):
            fdt = fd_pool.tile([in_dim, JB * P], BF16, tag="fd")
            nc.sync.dma_start(fdt[:], fd[:, t * JB * P:(t + 1) * JB * P])

            # zsT[p, d, j]: d-major per-edge [z | s2 | d2] so the j-axis
            # (reduced later) is innermost
            zsT = zs_pool.tile([P, OC * JB], BF16, tag="zs")
            zsT3 = zsT[:].rearrange("p (d j) -> p d j", j=JB)
            j0 = 0
            for b, nj in enumerate(bank_js):
                ps = ps_pool.tile([P, nj * OC], F32, tag=f"ps{b}")
                for q in range(nj):
                    j = j0 + q
                    nc.tensor.matmul(out=ps[:, q * OC:(q + 1) * OC],
                                     lhsT=fdt[:, j * P:(j + 1) * P],
                                     rhs=R_sb[:], start=True, stop=True)
                # evacuate transposed (d-major) + f32 -> bf16
                ps3 = ps[:].rearrange("p (j d) -> p j d", d=OC)
                nc.scalar.activation(out=zsT3[:, :, j0:j0 + nj],
                                     in_=ps3.transpose([0, 2, 1]),
                                     func=ACT.Copy)
                j0 += nj

            # e' = Lrelu(s2 + d2); sumE = sum_j e'  (free accumulator)
            e = sm_pool.tile([P, DEG], F32, tag="e")
            sumE = sm_pool.tile([P, 1], F32, tag="sumE")
            s2v = zsT[:][:, out_dim * JB:out_dim * JB + DEG]
            d2v = zsT[:][:, (out_dim + 1) * JB + DEG:(out_dim + 1) * JB + DEG + 1]
            nc.scalar.activation(out=e[:], in_=s2v, func=ACT.Lrelu,
                                 bias=d2v, scale=1.0, alpha=NEG_SLOPE,
                                 accum_out=sumE[:])
            # D[p,j,k] = e'_j - e'_k
            D = D_pool.tile([P, DEG * DEG], BF16, tag="D")
            D3 = D[:].rearrange("p (j k) -> p j k", k=DEG)
            nc.vector.tensor_tensor(
                out=D3, in0=e[:].unsqueeze(2).broadcast_to([P, DEG, DEG]),
                in1=e[:].unsqueeze(1).broadcast_to([P, DEG, DEG]),
                op=ALU.subtract)
            # A_j = sum_k |D|
            A = sm_pool.tile([P, DEG], F32, tag="A")
            nc.vector.tensor_reduce(out=A[:], in_=D3, axis=AXL.X, op=ALU.add,
                                    apply_absolute_value=True)
            # alpha = A + DEG*e' - sumE   (bf16 for the weighted sum)
            al0 = sm_pool.tile([P, DEG], F32, tag="al0")
            nc.vector.tensor_scalar(out=al0[:], in0=e[:], scalar1=float(DEG),
                                    scalar2=sumE[:], op0=ALU.mult,
                                    op1=ALU.subtract)
            alpha = sm_pool.tile([P, DEG], BF16, tag="alpha")
            nc.vector.tensor_tensor(out=alpha[:], in0=al0[:], in1=A[:],
                                    op=ALU.add)
            # alpha broadcast to [P, 64, 32] (d-major; Act copy)
            al64 = pr_pool.tile([P, out_dim * DEG], BF16, tag="al64")
            al64v = al64[:].rearrange("p (d j) -> p d j", j=DEG)
            nc.scalar.activation(
                out=al64v,
                in_=alpha[:].unsqueeze(1).broadcast_to([P, out_dim, DEG]),
                func=ACT.Copy)
            # prod[p,d,j] = z[p,d,j] * alpha[p,j]; fold j 32->16->8; reduce
            prod = pr_pool.tile([P, out_dim * DEG], BF16, tag="prod")
            prod3 = prod[:].rearrange("p (d j) -> p d j", j=DEG)
            nc.vector.tensor_tensor(out=prod3, in0=zsT3[:, 0:out_dim, 0:DEG],
                                    in1=al64v, op=ALU.mult)
            ph = pr_pool.tile([P, out_dim * (DEG // 2)], BF16, tag="ph")
            ph3 = ph[:].rearrange("p (d j) -> p d j", j=DEG // 2)
            nc.vector.tensor_tensor(out=ph3, in0=prod3[:, :, 0:DEG // 2],
                                    in1=prod3[:, :, DEG // 2:DEG], op=ALU.add)
            pq = pr_pool.tile([P, out_dim * (DEG // 4)], BF16, tag="pq")
            pq3 = pq[:].rearrange("p (d j) -> p d j", j=DEG // 4)
            nc.vector.tensor_tensor(out=pq3, in0=ph3[:, :, 0:DEG // 4],
                                    in1=ph3[:, :, DEG // 4:DEG // 2],
                                    op=ALU.add)
            if t % HGRP == 0:
                hbuf = h_pool.tile([P, HGRP * out_dim], F32, tag="hbuf")
            s = t % HGRP
            nc.vector.tensor_reduce(out=hbuf[:, s * out_dim:(s + 1) * out_dim],
                                    in_=pq3, axis=AXL.X, op=ALU.add)
            if t % HGRP == HGRP - 1 or t == ntiles - 1:
                g0 = (t // HGRP) * HGRP
                w = (t - g0 + 1) * out_dim
                nc.sync.dma_start(out=h[:, g0 * out_dim:g0 * out_dim + w],
                                  in_=hbuf[:, 0:w])

    nc.compile()
    return nc


def prep_inputs(feature, src_idx, fc_weight, attn_weight, ncores=NCORES):
    """Host-side sharding/layout prep -> per-core input maps."""
    import ml_dtypes

    bf16 = ml_dtypes.bfloat16
    feature = np.asarray(feature, dtype=np.float32)
    src = np.asarray(src_idx).astype(np.int64)
    fcw = np.asarray(fc_weight, dtype=np.float32)
    aw = np.asarray(attn_weight, dtype=np.float32)
    n, in_dim = feature.shape
    out_dim = fcw.shape[1]
    deg = src.shape[1]
    pn = PN
    ntiles = pn // P

    featT = np.ascontiguousarray(feature.T).astype(bf16)
    fcb = fcw.astype(bf16)
    fcT = np.ascontiguousarray(fcw.T).astype(bf16)
    attn2 = np.ascontiguousarray(
        np.stack([0.5 * aw[:out_dim, 0], 0.5 * aw[out_dim:, 0]],
                 axis=1)).astype(bf16)

    # padded per-core dest rows (cores own [c*pn, (c+1)*pn); rows >= n are
    # dummies whose outputs are discarded)
    src_pad = np.zeros((ncores * pn, deg), dtype=np.int64)
    src_pad[:n] = src
    featT_pad = np.zeros((in_dim, ncores * pn), dtype=bf16)
    featT_pad[:, :n] = featT

    in_maps = []
    for c in range(ncores):
        rows = np.arange(c * pn, (c + 1) * pn)
        cols = src_pad[c * pn:(c + 1) * pn]                 # [pn, deg]
        idx = np.concatenate([cols, rows[:, None]], axis=1)  # [pn, 33]
        # col order within a tile: j-major then p  -> (t, j, p)
        idx = idx.reshape(ntiles, P, JB).transpose(0, 2, 1).reshape(-1)
        fdc = np.ascontiguousarray(featT_pad[:, idx])
        in_maps.append({"fd": fdc, "fc": fcb, "fcT": fcT, "attn2": attn2})
    return in_maps, pn


_prog_cache = {}


def kernel(feature, src_idx, fc_weight, attn_weight):
    from concourse.bass_utils import run_bass_kernel_spmd

    in_maps, pn = prep_inputs(feature, src_idx, fc_weight, attn_weight)
    key = ("v6", feature.shape, pn)
    if key not in _prog_cache:
        _prog_cache[key] = build_program(pn=pn)
    nc = _prog_cache[key]
    res = run_bass_kernel_spmd(nc, in_maps, list(range(NCORES)))
    n = feature.shape[0]
    ntiles = pn // P
    h = np.zeros((NCORES * pn, OUT_DIM), dtype=np.float32)
    for c in range(NCORES):
        hc = np.asarray(res.results[c]["h"]).astype(np.float32)
        # h DRAM layout [128 p, ntiles*64] -> rows t*128+p
        hc = hc.reshape(P, ntiles, OUT_DIM).transpose(1, 0, 2).reshape(pn,
                                                                       OUT_DIM)
        h[c * pn:(c + 1) * pn] = hc
    return np.ascontiguousarray(h[:n], dtype=np.float32)


# revision 9
# speedup vs baseline: 1.9145x; 1.1934x over previous
"""Trainium2 Bass kernel for nn_CGATLayer (GNN message passing).

Algorithm (matches reference):
    z = feature @ fc_weight                      # [N, D]
    s = z @ attn[:D];  d = z @ attn[D:]          # per-node scalars
    e[n,j]   = leaky_relu(s[src[n,j]] + d[n])
    alpha[n,j] = sum_k relu(e[n,j] - e[n,k])
    h[n]     = sum_j alpha[n,j] * z[src[n,j]]

Device strategy (8 NeuronCores, SPMD single program):
  im2col formulation: instead of computing z per node and gathering rows
  per edge (DMA-descriptor bound: one descriptor per edge), the host ships
  the feature matrix with columns replicated in per-edge order
  (featdup[:, (t, j, p)] = feature[src[t*128+p, j]].T, j=32 = the dest's
  own feature column).  Each dest tile then needs 33 PE matmuls
  [128x128]x[128x66] against R = [fc | 0.5*fc@a1 | 0.5*fc@a2] to produce
  per-edge [z | s2 | d2] directly in PSUM -- per-edge data delivery rides
  on contiguous full-bandwidth DMA + the idle PE array instead of 200k
  512-byte gather descriptors.

  Using e' = e/2 (positive homogeneity of leaky_relu/relu):
      alpha = sum_k |e'_j - e'_k| + DEG*e'_j - sum_k e'_k
  so the pairwise clamp reduction is one abs-reduce.

  Per tile: 33 matmuls (5 PSUM banks) -> Act evacuates each bank
  transposed/bf16 into zsT [128, 66, 33] (d-major so the j-reduction is
  innermost) -> Act e' = Lrelu(s2 + d2) with free row-sum accumulator ->
  DVE pairwise diff (bf16 out) -> Pool abs-reduce -> DVE alpha, alpha
  broadcast to [128, 64, 32] (4x tensor_copy), product, one fold ->
  Pool reduces [128, 64, 16] -> h slot; h written to DRAM every 8 tiles
  in a partition-major layout so each partition's rows are contiguous.
"""

from contextlib import ExitStack

import numpy as np

import concourse.bass as bass
import concourse.bacc as bacc
import concourse.tile as tile
from concourse import mybir

F32 = mybir.dt.float32
BF16 = mybir.dt.bfloat16
ALU = mybir.AluOpType
AXL = mybir.AxisListType
ACT = mybir.ActivationFunctionType

N, DEG, IN_DIM, OUT_DIM = 50000, 32, 128, 64
NCORES = 8
NEG_SLOPE = 0.01
P = 128
PN = 6272                    # dest rows per core (49 tiles of 128)
NTILES = PN // P
JB = DEG + 1                 # 32 edge blocks + 1 own-feature block (d2)
OC = OUT_DIM + 2             # matmul output cols: z(64) | s2 | d2
HGRP = 8                     # tiles batched per h write


def build_program(pn=PN, in_dim=IN_DIM, out_dim=OUT_DIM, ncores=NCORES):
    ntiles = pn // P
    bank_js = [7, 7, 7, 7, 5]            # j-blocks per PSUM bank (sum=33)
    nh = (ntiles + HGRP - 1) // HGRP

    nc = bacc.Bacc("TRN2", num_devices=ncores)
    fd = nc.declare_dram_parameter("fd", [in_dim, ntiles * JB * P], BF16,
                                   isOutput=False)
    fc = nc.declare_dram_parameter("fc", [in_dim, out_dim], BF16, isOutput=False)
    fcT = nc.declare_dram_parameter("fcT", [out_dim, in_dim], BF16,
                                    isOutput=False)
    attn2 = nc.declare_dram_parameter("attn2", [out_dim, 2], BF16,
                                      isOutput=False)
    h = nc.declare_dram_parameter("h", [P, ntiles * out_dim], F32,
                                  isOutput=True)

    with tile.TileContext(nc) as tc, ExitStack() as ctx:
        const_pool = ctx.enter_context(tc.tile_pool(name="const", bufs=1))
        fd_pool = ctx.enter_context(tc.tile_pool(name="fd", bufs=3))
        ps_pool = ctx.enter_context(tc.tile_pool(name="ps", bufs=1,
                                                 space="PSUM"))
        zs_pool = ctx.enter_context(tc.tile_pool(name="zs", bufs=3))
        sm_pool = ctx.enter_context(tc.tile_pool(name="sm", bufs=3))
        D_pool = ctx.enter_context(tc.tile_pool(name="Dp", bufs=3))
        pr_pool = ctx.enter_context(tc.tile_pool(name="pr", bufs=3))
        h_pool = ctx.enter_context(tc.tile_pool(name="hp", bufs=2))

        # ---- weight prep: R = [fc | 0.5*fc@a1 | 0.5*fc@a2]  [in_dim, 66] bf16
        # (attn2 is pre-scaled by 0.5 on the host for both columns)
        fc_sb = const_pool.tile([in_dim, out_dim], BF16)
        nc.sync.dma_start(fc_sb[:], fc[:])
        fcT_sb = const_pool.tile([out_dim, in_dim], BF16)
        nc.sync.dma_start(fcT_sb[:], fcT[:])
        attn2_sb = const_pool.tile([out_dim, 2], BF16)
        nc.sync.dma_start(attn2_sb[:], attn2[:])
        R_sb = const_pool.tile([in_dim, OC], BF16)
        wsd_ps = ps_pool.tile([in_dim, 2], F32, tag="psW")
        nc.tensor.matmul(out=wsd_ps[:], lhsT=fcT_sb[:], rhs=attn2_sb[:],
                         start=True, stop=True)
        nc.vector.tensor_copy(out=R_sb[:, 0:out_dim], in_=fc_sb[:])
        nc.vector.tensor_copy(out=R_sb[:, out_dim:OC], in_=wsd_ps[:])

        hbuf = None
        for t in range(ntiles):
            fdt = fd_pool.tile([in_dim, JB * P], BF16, tag="fd")
            nc.sync.dma_start(fdt[:], fd[:, t * JB * P:(t + 1) * JB * P])

            # zsT[p, d, j]: d-major per-edge [z | s2 | d2] so the j-axis
            # (reduced later) is innermost
            zsT = zs_pool.tile([P, OC * JB], BF16, tag="zs")
            zsT3 = zsT[:].rearrange("p (d j) -> p d j", j=JB)
            j0 = 0
            for b, nj in enumerate(bank_js):
                ps = ps_pool.tile([P, nj * OC], F32, tag=f"ps{b}")
                for q in range(nj):
                    j = j0 + q
                    nc.tensor.matmul(out=ps[:, q * OC:(q + 1) * OC],
                                     lhsT=fdt[:, j * P:(j + 1) * P],
                                     rhs=R_sb[:], start=True, stop=True)
                # evacuate transposed (d-major) + f32 -> bf16
                ps3 = ps[:].rearrange("p (j d) -> p j d", d=OC)
                nc.scalar.activation(out=zsT3[:, :, j0:j0 + nj],
                                     in_=ps3.transpose([0, 2, 1]),
                                     func=ACT.Copy)
                j0 += nj

            # e' = Lrelu(s2 + d2); sumE = sum_j e'  (free accumulator)
            e = sm_pool.tile([P, DEG], F32, tag="e")
            sumE = sm_pool.tile([P, 1], F32, tag="sumE")
            s2v = zsT[:][:, out_dim * JB:out_dim * JB + DEG]
            d2v = zsT[:][:, (out_dim + 1) * JB + DEG:(out_dim + 1) * JB + DEG + 1]
            nc.scalar.activation(out=e[:], in_=s2v, func=ACT.Lrelu,
                                 bias=d2v, scale=1.0, alpha=NEG_SLOPE,
                                 accum_out=sumE[:])
            # D[p,j,k] = e'_j - e'_k   (Pool)
            D = D_pool.tile([P, DEG * DEG], BF16, tag="D")
            D3 = D[:].rearrange("p (j k) -> p j k", k=DEG)
            nc.gpsimd.tensor_tensor(
                out=D3, in0=e[:].unsqueeze(2).broadcast_to([P, DEG, DEG]),
                in1=e[:].unsqueeze(1).broadcast_to([P, DEG, DEG]),
                op=ALU.subtract)
            # A_j = sum_k |D|
            A = sm_pool.tile([P, DEG], F32, tag="A")
            nc.vector.tensor_reduce(out=A[:], in_=D3, axis=AXL.X, op=ALU.add,
                                    apply_absolute_value=True)
            # alpha = A + DEG*e' - sumE   (bf16 for the weighted sum)
            al0 = sm_pool.tile([P, DEG], F32, tag="al0")
            nc.vector.tensor_scalar(out=al0[:], in0=e[:], scalar1=float(DEG),
                                    scalar2=sumE[:], op0=ALU.mult,
                                    op1=ALU.subtract)
            alpha = sm_pool.tile([P, DEG], BF16, tag="alpha")
            nc.gpsimd.tensor_tensor(out=alpha[:], in0=al0[:], in1=A[:],
                                    op=ALU.add)
            # prod[p,d,j] = z[p,d,j] * alpha[p,j] -- alpha rides as a
            # broadcast view (middle-dim stride 0, packed last => 2x DVE)
            prod = pr_pool.tile([P, out_dim * DEG], BF16, tag="prod")
            prod3 = prod[:].rearrange("p (d j) -> p d j", j=DEG)
            nc.vector.tensor_tensor(
                out=prod3, in0=zsT3[:, 0:out_dim, 0:DEG],
                in1=alpha[:].unsqueeze(1).broadcast_to([P, out_dim, DEG]),
                op=ALU.mult)
            # fold j 32->16 (DVE) ->8 (Pool), then reduce 8->1 (DVE)
            ph = pr_pool.tile([P, out_dim * (DEG // 2)], BF16, tag="ph")
            ph3 = ph[:].rearrange("p (d j) -> p d j", j=DEG // 2)
            nc.vector.tensor_tensor(out=ph3, in0=prod3[:, :, 0:DEG // 2],
                                    in1=prod3[:, :, DEG // 2:DEG], op=ALU.add)
            pq = pr_pool.tile([P, out_dim * (DEG // 4)], BF16, tag="pq")
            pq3 = pq[:].rearrange("p (d j) -> p d j", j=DEG // 4)
            nc.gpsimd.tensor_tensor(out=pq3, in0=ph3[:, :, 0:DEG // 4],
                                    in1=ph3[:, :, DEG // 4:DEG // 2],
                                    op=ALU.add)
            if t % HGRP == 0:
                hbuf = h_pool.tile([P, HGRP * out_dim], F32, tag="hbuf")
            s = t % HGRP
            nc.vector.tensor_reduce(out=hbuf[:, s * out_dim:(s + 1) * out_dim],
                                    in_=pq3, axis=AXL.X, op=ALU.add)
            if t % HGRP == HGRP - 1 or t == ntiles - 1:
                g0 = (t // HGRP) * HGRP
                w = (t - g0 + 1) * out_dim
                nc.sync.dma_start(out=h[:, g0 * out_dim:g0 * out_dim + w],
                                  in_=hbuf[:, 0:w])

    nc.compile()
    return nc


def prep_inputs(feature, src_idx, fc_weight, attn_weight, ncores=NCORES):
    """Host-side sharding/layout prep -> per-core input maps."""
    import ml_dtypes

    bf16 = ml_dtypes.bfloat16
    feature = np.asarray(feature, dtype=np.float32)
    src = np.asarray(src_idx).astype(np.int64)
    fcw = np.asarray(fc_weight, dtype=np.float32)
    aw = np.asarray(attn_weight, dtype=np.float32)
    n, in_dim = feature.shape
    out_dim = fcw.shape[1]
    deg = src.shape[1]
    pn = PN
    ntiles = pn // P

    featT = np.ascontiguousarray(feature.T).astype(bf16)
    fcb = fcw.astype(bf16)
    fcT = np.ascontiguousarray(fcw.T).astype(bf16)
    attn2 = np.ascontiguousarray(
        np.stack([0.5 * aw[:out_dim, 0], 0.5 * aw[out_dim:, 0]],
                 axis=1)).astype(bf16)

    # padded per-core dest rows (cores own [c*pn, (c+1)*pn); rows >= n are
    # dummies whose outputs are discarded)
    src_pad = np.zeros((ncores * pn, deg), dtype=np.int64)
    src_pad[:n] = src
    featT_pad = np.zeros((in_dim, ncores * pn), dtype=bf16)
    featT_pad[:, :n] = featT

    in_maps = []
    for c in range(ncores):
        rows = np.arange(c * pn, (c + 1) * pn)
        cols = src_pad[c * pn:(c + 1) * pn]                 # [pn, deg]
        idx = np.concatenate([cols, rows[:, None]], axis=1)  # [pn, 33]
        # col order within a tile: j-major then p  -> (t, j, p)
        idx = idx.reshape(ntiles, P, JB).transpose(0, 2, 1).reshape(-1)
        fdc = np.ascontiguousarray(featT_pad[:, idx])
        in_maps.append({"fd": fdc, "fc": fcb, "fcT": fcT, "attn2": attn2})
    return in_maps, pn


_prog_cache = {}


def kernel(feature, src_idx, fc_weight, attn_weight):
    from concourse.bass_utils import run_bass_kernel_spmd

    in_maps, pn = prep_inputs(feature, src_idx, fc_weight, attn_weight)
    key = ("v6", feature.shape, pn)
    if key not in _prog_cache:
        _prog_cache[key] = build_program(pn=pn)
    nc = _prog_cache[key]
    res = run_bass_kernel_spmd(nc, in_maps, list(range(NCORES)))
    n = feature.shape[0]
    ntiles = pn // P
    h = np.zeros((NCORES * pn, OUT_DIM), dtype=np.float32)
    for c in range(NCORES):
        hc = np.asarray(res.results[c]["h"]).astype(np.float32)
        # h DRAM layout [128 p, ntiles*64] -> rows t*128+p
        hc = hc.reshape(P, ntiles, OUT_DIM).transpose(1, 0, 2).reshape(pn,
                                                                       OUT_DIM)
        h[c * pn:(c + 1) * pn] = hc
    return np.ascontiguousarray(h[:n], dtype=np.float32)


# revision 10
# speedup vs baseline: 1.9700x; 1.0290x over previous
"""Trainium2 Bass kernel for nn_CGATLayer (GNN message passing).

Algorithm (matches reference):
    z = feature @ fc_weight                      # [N, D]
    s = z @ attn[:D];  d = z @ attn[D:]          # per-node scalars
    e[n,j]   = leaky_relu(s[src[n,j]] + d[n])
    alpha[n,j] = sum_k relu(e[n,j] - e[n,k])
    h[n]     = sum_j alpha[n,j] * z[src[n,j]]

Device strategy (8 NeuronCores, SPMD single program):
  im2col formulation: instead of computing z per node and gathering rows
  per edge (DMA-descriptor bound: one descriptor per edge), the host ships
  the feature matrix with columns replicated in per-edge order
  (featdup[:, (t, j, p)] = feature[src[t*128+p, j]].T, j=32 = the dest's
  own feature column).  Each dest tile then needs 33 PE matmuls
  [128x128]x[128x66] against R = [fc | 0.5*fc@a1 | 0.5*fc@a2] to produce
  per-edge [z | s2 | d2] directly in PSUM -- per-edge data delivery rides
  on contiguous full-bandwidth DMA + the idle PE array instead of 200k
  512-byte gather descriptors.

  Using e' = e/2 (positive homogeneity of leaky_relu/relu):
      alpha = sum_k |e'_j - e'_k| + DEG*e'_j - sum_k e'_k
  so the pairwise clamp reduction is one abs-reduce.

  Per tile: 33 matmuls (5 PSUM banks) -> Act evacuates each bank
  transposed/bf16 into zsT [128, 66, 33] (d-major so the j-reduction is
  innermost) -> Act e' = Lrelu(s2 + d2) with free row-sum accumulator ->
  DVE pairwise diff (bf16 out) -> Pool abs-reduce -> DVE alpha, alpha
  broadcast to [128, 64, 32] (4x tensor_copy), product, one fold ->
  Pool reduces [128, 64, 16] -> h slot; h written to DRAM every 8 tiles
  in a partition-major layout so each partition's rows are contiguous.
"""

from contextlib import ExitStack

import numpy as np

import concourse.bass as bass
import concourse.bacc as bacc
import concourse.tile as tile
from concourse import mybir

F32 = mybir.dt.float32
BF16 = mybir.dt.bfloat16
ALU = mybir.AluOpType
AXL = mybir.AxisListType
ACT = mybir.ActivationFunctionType

N, DEG, IN_DIM, OUT_DIM = 50000, 32, 128, 64
NCORES = 8
NEG_SLOPE = 0.01
P = 128
PN = 6272                    # dest rows per core (49 tiles of 128)
NTILES = PN // P
JB = DEG + 1                 # 32 edge blocks + 1 own-feature block (d2)
OC = OUT_DIM + 2             # matmul output cols: z(64) | s2 | d2
HGRP = 8                     # tiles batched per h write


def build_program(pn=PN, in_dim=IN_DIM, out_dim=OUT_DIM, ncores=NCORES):
    ntiles = pn // P
    bank_js = [7, 7, 7, 7, 5]            # j-blocks per PSUM bank (sum=33)
    nh = (ntiles + HGRP - 1) // HGRP

    nc = bacc.Bacc("TRN2", num_devices=ncores)
    fd = nc.declare_dram_parameter("fd", [in_dim, ntiles * JB * P], BF16,
                                   isOutput=False)
    fc = nc.declare_dram_parameter("fc", [in_dim, out_dim], BF16, isOutput=False)
    fcT = nc.declare_dram_parameter("fcT", [out_dim, in_dim], BF16,
                                    isOutput=False)
    attn2 = nc.declare_dram_parameter("attn2", [out_dim, 2], BF16,
                                      isOutput=False)
    h = nc.declare_dram_parameter("h", [P, ntiles * out_dim], F32,
                                  isOutput=True)

    with tile.TileContext(nc) as tc, ExitStack() as ctx:
        const_pool = ctx.enter_context(tc.tile_pool(name="const", bufs=1))
        fd_pool = ctx.enter_context(tc.tile_pool(name="fd", bufs=3))
        ps_pool = ctx.enter_context(tc.tile_pool(name="ps", bufs=1,
                                                 space="PSUM"))
        zs_pool = ctx.enter_context(tc.tile_pool(name="zs", bufs=3))
        sm_pool = ctx.enter_context(tc.tile_pool(name="sm", bufs=3))
        D_pool = ctx.enter_context(tc.tile_pool(name="Dp", bufs=3))
        pr_pool = ctx.enter_context(tc.tile_pool(name="pr", bufs=3))
        h_pool = ctx.enter_context(tc.tile_pool(name="hp", bufs=2))

        # ---- weight prep: R = [fc | 0.5*fc@a1 | 0.5*fc@a2]  [in_dim, 66] bf16
        # (attn2 is pre-scaled by 0.5 on the host for both columns)
        fc_sb = const_pool.tile([in_dim, out_dim], BF16)
        nc.sync.dma_start(fc_sb[:], fc[:])
        fcT_sb = const_pool.tile([out_dim, in_dim], BF16)
        nc.sync.dma_start(fcT_sb[:], fcT[:])
        attn2_sb = const_pool.tile([out_dim, 2], BF16)
        nc.sync.dma_start(attn2_sb[:], attn2[:])
        R_sb = const_pool.tile([in_dim, OC], BF16)
        wsd_ps = ps_pool.tile([in_dim, 2], F32, tag="psW")
        nc.tensor.matmul(out=wsd_ps[:], lhsT=fcT_sb[:], rhs=attn2_sb[:],
                         start=True, stop=True)
        nc.vector.tensor_copy(out=R_sb[:, 0:out_dim], in_=fc_sb[:])
        nc.vector.tensor_copy(out=R_sb[:, out_dim:OC], in_=wsd_ps[:])

        def stage1(t):
            """DMA + matmuls + evac + e' + D + al0 for tile t."""
            fdt = fd_pool.tile([in_dim, JB * P], BF16, tag="fd")
            nc.sync.dma_start(fdt[:], fd[:, t * JB * P:(t + 1) * JB * P])

            # zsT[p, d, j]: d-major per-edge [z | s2 | d2] so the j-axis
            # (reduced later) is innermost
            zsT = zs_pool.tile([P, OC * JB], BF16, tag="zs")
            zsT3 = zsT[:].rearrange("p (d j) -> p d j", j=JB)
            j0 = 0
            for b, nj in enumerate(bank_js):
                ps = ps_pool.tile([P, nj * OC], F32, tag=f"ps{b}")
                for q in range(nj):
                    j = j0 + q
                    nc.tensor.matmul(out=ps[:, q * OC:(q + 1) * OC],
                                     lhsT=fdt[:, j * P:(j + 1) * P],
                                     rhs=R_sb[:], start=True, stop=True)
                # evacuate transposed (d-major) + f32 -> bf16
                ps3 = ps[:].rearrange("p (j d) -> p j d", d=OC)
                nc.scalar.activation(out=zsT3[:, :, j0:j0 + nj],
                                     in_=ps3.transpose([0, 2, 1]),
                                     func=ACT.Copy)
                j0 += nj

            # e' = Lrelu(s2 + d2); sumE = sum_j e'  (free accumulator)
            e = sm_pool.tile([P, DEG], F32, tag="e")
            sumE = sm_pool.tile([P, 1], F32, tag="sumE")
            s2v = zsT[:][:, out_dim * JB:out_dim * JB + DEG]
            d2v = zsT[:][:, (out_dim + 1) * JB + DEG:(out_dim + 1) * JB + DEG + 1]
            nc.scalar.activation(out=e[:], in_=s2v, func=ACT.Lrelu,
                                 bias=d2v, scale=1.0, alpha=NEG_SLOPE,
                                 accum_out=sumE[:])
            # D[p,j,k] = e'_j - e'_k   (Pool)
            D = D_pool.tile([P, DEG * DEG], BF16, tag="D")
            D3 = D[:].rearrange("p (j k) -> p j k", k=DEG)
            nc.gpsimd.tensor_tensor(
                out=D3, in0=e[:].unsqueeze(2).broadcast_to([P, DEG, DEG]),
                in1=e[:].unsqueeze(1).broadcast_to([P, DEG, DEG]),
                op=ALU.subtract)
            # al0 = DEG*e' - sumE
            al0 = sm_pool.tile([P, DEG], F32, tag="al0")
            nc.vector.tensor_scalar(out=al0[:], in0=e[:], scalar1=float(DEG),
                                    scalar2=sumE[:], op0=ALU.mult,
                                    op1=ALU.subtract)
            return zsT3, D3, al0

        def stage2(t, zsT3, D3, al0):
            """A + alpha + weighted sum for tile t (one pipeline slot old)."""
            nonlocal hbuf
            A = sm_pool.tile([P, DEG], F32, tag="A")
            nc.vector.tensor_reduce(out=A[:], in_=D3, axis=AXL.X, op=ALU.add,
                                    apply_absolute_value=True)
            # alpha = A + al0   (bf16 for the weighted sum)
            alpha = sm_pool.tile([P, DEG], BF16, tag="alpha")
            nc.gpsimd.tensor_tensor(out=alpha[:], in0=al0[:], in1=A[:],
                                    op=ALU.add)
            # prod[p,d,j] = z[p,d,j] * alpha[p,j] -- alpha rides as a
            # broadcast view (middle-dim stride 0, packed last => 2x DVE)
            prod = pr_pool.tile([P, out_dim * DEG], BF16, tag="prod")
            prod3 = prod[:].rearrange("p (d j) -> p d j", j=DEG)
            nc.vector.tensor_tensor(
                out=prod3, in0=zsT3[:, 0:out_dim, 0:DEG],
                in1=alpha[:].unsqueeze(1).broadcast_to([P, out_dim, DEG]),
                op=ALU.mult)
            # fold j 32->16 (DVE) ->8 (Pool), then reduce 8->1 (DVE)
            ph = pr_pool.tile([P, out_dim * (DEG // 2)], BF16, tag="ph")
            ph3 = ph[:].rearrange("p (d j) -> p d j", j=DEG // 2)
            nc.vector.tensor_tensor(out=ph3, in0=prod3[:, :, 0:DEG // 2],
                                    in1=prod3[:, :, DEG // 2:DEG], op=ALU.add)
            pq = pr_pool.tile([P, out_dim * (DEG // 4)], BF16, tag="pq")
            pq3 = pq[:].rearrange("p (d j) -> p d j", j=DEG // 4)
            nc.gpsimd.tensor_tensor(out=pq3, in0=ph3[:, :, 0:DEG // 4],
                                    in1=ph3[:, :, DEG // 4:DEG // 2],
                                    op=ALU.add)
            if t % HGRP == 0:
                hbuf = h_pool.tile([P, HGRP * out_dim], F32, tag="hbuf")
            s = t % HGRP
            nc.vector.tensor_reduce(out=hbuf[:, s * out_dim:(s + 1) * out_dim],
                                    in_=pq3, axis=AXL.X, op=ALU.add)
            if t % HGRP == HGRP - 1 or t == ntiles - 1:
                g0 = (t // HGRP) * HGRP
                w = (t - g0 + 1) * out_dim
                nc.sync.dma_start(out=h[:, g0 * out_dim:g0 * out_dim + w],
                                  in_=hbuf[:, 0:w])

        hbuf = None
        prev = None
        for t in range(ntiles + 1):
            cur = stage1(t) if t < ntiles else None
            if prev is not None:
                stage2(t - 1, *prev)
            prev = cur

    nc.compile()
    return nc


def prep_inputs(feature, src_idx, fc_weight, attn_weight, ncores=NCORES):
    """Host-side sharding/layout prep -> per-core input maps."""
    import ml_dtypes

    bf16 = ml_dtypes.bfloat16
    feature = np.asarray(feature, dtype=np.float32)
    src = np.asarray(src_idx).astype(np.int64)
    fcw = np.asarray(fc_weight, dtype=np.float32)
    aw = np.asarray(attn_weight, dtype=np.float32)
    n, in_dim = feature.shape
    out_dim = fcw.shape[1]
    deg = src.shape[1]
    pn = PN
    ntiles = pn // P

    featT = np.ascontiguousarray(feature.T).astype(bf16)
    fcb = fcw.astype(bf16)
    fcT = np.ascontiguousarray(fcw.T).astype(bf16)
    attn2 = np.ascontiguousarray(
        np.stack([0.5 * aw[:out_dim, 0], 0.5 * aw[out_dim:, 0]],
                 axis=1)).astype(bf16)

    # padded per-core dest rows (cores own [c*pn, (c+1)*pn); rows >= n are
    # dummies whose outputs are discarded)
    src_pad = np.zeros((ncores * pn, deg), dtype=np.int64)
    src_pad[:n] = src
    featT_pad = np.zeros((in_dim, ncores * pn), dtype=bf16)
    featT_pad[:, :n] = featT

    in_maps = []
    for c in range(ncores):
        rows = np.arange(c * pn, (c + 1) * pn)
        cols = src_pad[c * pn:(c + 1) * pn]                 # [pn, deg]
        idx = np.concatenate([cols, rows[:, None]], axis=1)  # [pn, 33]
        # col order within a tile: j-major then p  -> (t, j, p)
        idx = idx.reshape(ntiles, P, JB).transpose(0, 2, 1).reshape(-1)
        fdc = np.ascontiguousarray(featT_pad[:, idx])
        in_maps.append({"fd": fdc, "fc": fcb, "fcT": fcT, "attn2": attn2})
    return in_maps, pn


_prog_cache = {}


def kernel(feature, src_idx, fc_weight, attn_weight):
    from concourse.bass_utils import run_bass_kernel_spmd

    in_maps, pn = prep_inputs(feature, src_idx, fc_weight, attn_weight)
    key = ("v6", feature.shape, pn)
    if key not in _prog_cache:
        _prog_cache[key] = build_program(pn=pn)
    nc = _prog_cache[key]
    res = run_bass_kernel_spmd(nc, in_maps, list(range(NCORES)))
    n = feature.shape[0]
    ntiles = pn // P
    h = np.zeros((NCORES * pn, OUT_DIM), dtype=np.float32)
    for c in range(NCORES):
        hc = np.asarray(res.results[c]["h"]).astype(np.float32)
        # h DRAM layout [128 p, ntiles*64] -> rows t*128+p
        hc = hc.reshape(P, ntiles, OUT_DIM).transpose(1, 0, 2).reshape(pn,
                                                                       OUT_DIM)
        h[c * pn:(c + 1) * pn] = hc
    return np.ascontiguousarray(h[:n], dtype=np.float32)


# revision 13
# speedup vs baseline: 2.1103x; 1.0712x over previous
"""Trainium2 Bass kernel for nn_CGATLayer (GNN message passing).

Algorithm (matches reference):
    z = feature @ fc_weight                      # [N, D]
    s = z @ attn[:D];  d = z @ attn[D:]          # per-node scalars
    e[n,j]   = leaky_relu(s[src[n,j]] + d[n])
    alpha[n,j] = sum_k relu(e[n,j] - e[n,k])
    h[n]     = sum_j alpha[n,j] * z[src[n,j]]

Device strategy (8 NeuronCores, SPMD single program):
  im2col formulation: instead of computing z per node and gathering rows
  per edge (DMA-descriptor bound: one descriptor per edge), the host ships
  the feature matrix with columns replicated in per-edge order
  (featdup[:, (t, j, p)] = feature[src[t*128+p, j]].T, j=32 = the dest's
  own feature column).  Each dest tile then needs 33 PE matmuls
  [128x128]x[128x66] against R = [fc | 0.5*fc@a1 | 0.5*fc@a2] to produce
  per-edge [z | s2 | d2] directly in PSUM -- per-edge data delivery rides
  on contiguous full-bandwidth DMA + the idle PE array instead of 200k
  512-byte gather descriptors.

  Using e' = e/2 (positive homogeneity of leaky_relu/relu):
      alpha = sum_k |e'_j - e'_k| + DEG*e'_j - sum_k e'_k
  so the pairwise clamp reduction is one abs-reduce.

  Per tile: 33 matmuls (5 PSUM banks) -> Act evacuates each bank
  transposed/bf16 into zsT [128, 66, 33] (d-major so the j-reduction is
  innermost) -> Act e' = Lrelu(s2 + d2) with free row-sum accumulator ->
  DVE pairwise diff (bf16 out) -> Pool abs-reduce -> DVE alpha, alpha
  broadcast to [128, 64, 32] (4x tensor_copy), product, one fold ->
  Pool reduces [128, 64, 16] -> h slot; h written to DRAM every 8 tiles
  in a partition-major layout so each partition's rows are contiguous.
"""

from contextlib import ExitStack

import numpy as np

import concourse.bass as bass
import concourse.bacc as bacc
import concourse.tile as tile
from concourse import mybir

F32 = mybir.dt.float32
BF16 = mybir.dt.bfloat16
ALU = mybir.AluOpType
AXL = mybir.AxisListType
ACT = mybir.ActivationFunctionType

N, DEG, IN_DIM, OUT_DIM = 50000, 32, 128, 64
NCORES = 8
NEG_SLOPE = 0.01
P = 128
PN = 6272                    # dest rows per core (49 tiles of 128)
NTILES = PN // P
JB = DEG + 1                 # 32 edge blocks + 1 own-feature block (d2)
OC = OUT_DIM + 2             # matmul output cols: z(64) | s2 | d2
HGRP = 8                     # tiles batched per h write


def build_program(pn=PN, in_dim=IN_DIM, out_dim=OUT_DIM, ncores=NCORES):
    ntiles = pn // P
    bank_js = [7, 7, 7, 7, 5]            # j-blocks per PSUM bank (sum=33)
    nh = (ntiles + HGRP - 1) // HGRP

    nc = bacc.Bacc("TRN2", num_devices=ncores)
    fd = nc.declare_dram_parameter("fd", [in_dim, ntiles * JB * P], BF16,
                                   isOutput=False)
    fc = nc.declare_dram_parameter("fc", [in_dim, out_dim], BF16, isOutput=False)
    fcT = nc.declare_dram_parameter("fcT", [out_dim, in_dim], BF16,
                                    isOutput=False)
    attn2 = nc.declare_dram_parameter("attn2", [out_dim, 2], BF16,
                                      isOutput=False)
    h = nc.declare_dram_parameter("h", [P, ntiles * out_dim], F32,
                                  isOutput=True)

    with tile.TileContext(nc) as tc, ExitStack() as ctx:
        const_pool = ctx.enter_context(tc.tile_pool(name="const", bufs=1))
        fd_pool = ctx.enter_context(tc.tile_pool(name="fd", bufs=3))
        ps_pool = ctx.enter_context(tc.tile_pool(name="ps", bufs=1,
                                                 space="PSUM"))
        zs_pool = ctx.enter_context(tc.tile_pool(name="zs", bufs=7))
        sm_pool = ctx.enter_context(tc.tile_pool(name="sm", bufs=6))
        D_pool = ctx.enter_context(tc.tile_pool(name="Dp", bufs=4))
        pr_pool = ctx.enter_context(tc.tile_pool(name="pr", bufs=4))
        h_pool = ctx.enter_context(tc.tile_pool(name="hp", bufs=2))

        # ---- weight prep: R = [fc | 0.5*fc@a1 | 0.5*fc@a2]  [in_dim, 66] bf16
        # (attn2 is pre-scaled by 0.5 on the host for both columns)
        fc_sb = const_pool.tile([in_dim, out_dim], BF16)
        nc.sync.dma_start(fc_sb[:], fc[:])
        fcT_sb = const_pool.tile([out_dim, in_dim], BF16)
        nc.sync.dma_start(fcT_sb[:], fcT[:])
        attn2_sb = const_pool.tile([out_dim, 2], BF16)
        nc.sync.dma_start(attn2_sb[:], attn2[:])
        R_sb = const_pool.tile([in_dim, OC], BF16)
        wsd_ps = ps_pool.tile([in_dim, 2], F32, tag="psW")
        nc.tensor.matmul(out=wsd_ps[:], lhsT=fcT_sb[:], rhs=attn2_sb[:],
                         start=True, stop=True)
        nc.vector.tensor_copy(out=R_sb[:, 0:out_dim], in_=fc_sb[:])
        nc.vector.tensor_copy(out=R_sb[:, out_dim:OC], in_=wsd_ps[:])

        st = {}

        def stage1(t):
            """DMA + matmuls + evac + e' for tile t."""
            fdt = fd_pool.tile([in_dim, JB * P], BF16, tag="fd")
            nc.sync.dma_start(fdt[:], fd[:, t * JB * P:(t + 1) * JB * P])

            # zsT[p, d, j]: d-major per-edge [z | s2 | d2] so the j-axis
            # (reduced later) is innermost
            zsT = zs_pool.tile([P, OC * JB], BF16, tag="zs")
            zsT3 = zsT[:].rearrange("p (d j) -> p d j", j=JB)
            j0 = 0
            for b, nj in enumerate(bank_js):
                ps = ps_pool.tile([P, nj * OC], F32, tag=f"ps{b}")
                for q in range(nj):
                    j = j0 + q
                    nc.tensor.matmul(out=ps[:, q * OC:(q + 1) * OC],
                                     lhsT=fdt[:, j * P:(j + 1) * P],
                                     rhs=R_sb[:], start=True, stop=True)
                # evacuate transposed (d-major) + f32 -> bf16
                ps3 = ps[:].rearrange("p (j d) -> p j d", d=OC)
                nc.scalar.activation(out=zsT3[:, :, j0:j0 + nj],
                                     in_=ps3.transpose([0, 2, 1]),
                                     func=ACT.Copy)
                j0 += nj

            # e' = Lrelu(s2 + d2); sumE = sum_j e'  (free accumulator)
            e = sm_pool.tile([P, DEG], F32, tag="e")
            sumE = sm_pool.tile([P, 1], F32, tag="sumE")
            s2v = zsT[:][:, out_dim * JB:out_dim * JB + DEG]
            d2v = zsT[:][:, (out_dim + 1) * JB + DEG:(out_dim + 1) * JB + DEG + 1]
            nc.scalar.activation(out=e[:], in_=s2v, func=ACT.Lrelu,
                                 bias=d2v, scale=1.0, alpha=NEG_SLOPE,
                                 accum_out=sumE[:])
            st[t] = {"zsT3": zsT3, "e": e, "sumE": sumE}

        def stage2(t):
            """D (Pool) + al0 (DVE), inputs one iteration old."""
            s = st[t]
            D = D_pool.tile([P, DEG * DEG], BF16, tag="D")
            D3 = D[:].rearrange("p (j k) -> p j k", k=DEG)
            e = s["e"]
            nc.gpsimd.tensor_tensor(
                out=D3, in0=e[:].unsqueeze(2).broadcast_to([P, DEG, DEG]),
                in1=e[:].unsqueeze(1).broadcast_to([P, DEG, DEG]),
                op=ALU.subtract)
            al0 = sm_pool.tile([P, DEG], F32, tag="al0")
            nc.vector.tensor_scalar(out=al0[:], in0=e[:], scalar1=float(DEG),
                                    scalar2=s["sumE"][:], op0=ALU.mult,
                                    op1=ALU.subtract)
            s["D3"], s["al0"] = D3, al0

        def stage3(t):
            """A = sum_k |D|  (DVE)."""
            s = st[t]
            A = sm_pool.tile([P, DEG], F32, tag="A")
            nc.vector.tensor_reduce(out=A[:], in_=s["D3"], axis=AXL.X,
                                    op=ALU.add, apply_absolute_value=True)
            s["A"] = A

        def stage4(t):
            """alpha = A + al0  (Pool; bf16 for the weighted sum)."""
            s = st[t]
            alpha = sm_pool.tile([P, DEG], BF16, tag="alpha")
            nc.gpsimd.tensor_tensor(out=alpha[:], in0=s["al0"][:],
                                    in1=s["A"][:], op=ALU.add)
            s["alpha"] = alpha

        def stage5(t):
            """prod + fold 32->16  (DVE); alpha rides as a broadcast view
            (middle-dim stride 0, packed last => 2x)."""
            s = st[t]
            prod = pr_pool.tile([P, out_dim * DEG], BF16, tag="prod")
            prod3 = prod[:].rearrange("p (d j) -> p d j", j=DEG)
            nc.vector.tensor_tensor(
                out=prod3, in0=s["zsT3"][:, 0:out_dim, 0:DEG],
                in1=s["alpha"][:].unsqueeze(1).broadcast_to(
                    [P, out_dim, DEG]),
                op=ALU.mult)
            ph = pr_pool.tile([P, out_dim * (DEG // 2)], BF16, tag="ph")
            ph3 = ph[:].rearrange("p (d j) -> p d j", j=DEG // 2)
            nc.vector.tensor_tensor(out=ph3, in0=prod3[:, :, 0:DEG // 2],
                                    in1=prod3[:, :, DEG // 2:DEG], op=ALU.add)
            s["ph3"] = ph3

        def stage6(t):
            """fold 16->8  (Pool)."""
            s = st[t]
            pq = pr_pool.tile([P, out_dim * (DEG // 4)], BF16, tag="pq")
            pq3 = pq[:].rearrange("p (d j) -> p d j", j=DEG // 4)
            nc.gpsimd.tensor_tensor(out=pq3, in0=s["ph3"][:, :, 0:DEG // 4],
                                    in1=s["ph3"][:, :, DEG // 4:DEG // 2],
                                    op=ALU.add)
            s["pq3"] = pq3

        def stage7(t):
            """reduce 8->1 into the h slot (DVE); h DMA every HGRP tiles."""
            nonlocal hbuf
            s = st.pop(t)
            if t % HGRP == 0:
                hbuf = h_pool.tile([P, HGRP * out_dim], F32, tag="hbuf")
            sl = t % HGRP
            nc.vector.tensor_reduce(
                out=hbuf[:, sl * out_dim:(sl + 1) * out_dim],
                in_=s["pq3"], axis=AXL.X, op=ALU.add)
            if t % HGRP == HGRP - 1 or t == ntiles - 1:
                g0 = (t // HGRP) * HGRP
                w = (t - g0 + 1) * out_dim
                nc.sync.dma_start(out=h[:, g0 * out_dim:g0 * out_dim + w],
                                  in_=hbuf[:, 0:w])

        hbuf = None
        phases = [stage1, stage2, stage3, stage4, stage5, stage6, stage7]
        for i in range(ntiles + len(phases) - 1):
            for k, phase in enumerate(phases):
                t = i - k
                if 0 <= t < ntiles:
                    phase(t)

    nc.compile()
    return nc


def prep_inputs(feature, src_idx, fc_weight, attn_weight, ncores=NCORES):
    """Host-side sharding/layout prep -> per-core input maps."""
    import ml_dtypes

    bf16 = ml_dtypes.bfloat16
    feature = np.asarray(feature, dtype=np.float32)
    src = np.asarray(src_idx).astype(np.int64)
    fcw = np.asarray(fc_weight, dtype=np.float32)
    aw = np.asarray(attn_weight, dtype=np.float32)
    n, in_dim = feature.shape
    out_dim = fcw.shape[1]
    deg = src.shape[1]
    pn = PN
    ntiles = pn // P

    featT = np.ascontiguousarray(feature.T).astype(bf16)
    fcb = fcw.astype(bf16)
    fcT = np.ascontiguousarray(fcw.T).astype(bf16)
    attn2 = np.ascontiguousarray(
        np.stack([0.5 * aw[:out_dim, 0], 0.5 * aw[out_dim:, 0]],
                 axis=1)).astype(bf16)

    # padded per-core dest rows (cores own [c*pn, (c+1)*pn); rows >= n are
    # dummies whose outputs are discarded)
    src_pad = np.zeros((ncores * pn, deg), dtype=np.int64)
    src_pad[:n] = src
    featT_pad = np.zeros((in_dim, ncores * pn), dtype=bf16)
    featT_pad[:, :n] = featT

    in_maps = []
    for c in range(ncores):
        rows = np.arange(c * pn, (c + 1) * pn)
        cols = src_pad[c * pn:(c + 1) * pn]                 # [pn, deg]
        idx = np.concatenate([cols, rows[:, None]], axis=1)  # [pn, 33]
        # col order within a tile: j-major then p  -> (t, j, p)
        idx = idx.reshape(ntiles, P, JB).transpose(0, 2, 1).reshape(-1)
        fdc = np.ascontiguousarray(featT_pad[:, idx])
        in_maps.append({"fd": fdc, "fc": fcb, "fcT": fcT, "attn2": attn2})
    return in_maps, pn


_prog_cache = {}


def kernel(feature, src_idx, fc_weight, attn_weight):
    from concourse.bass_utils import run_bass_kernel_spmd

    in_maps, pn = prep_inputs(feature, src_idx, fc_weight, attn_weight)
    key = ("v6", feature.shape, pn)
    if key not in _prog_cache:
        _prog_cache[key] = build_program(pn=pn)
    nc = _prog_cache[key]
    res = run_bass_kernel_spmd(nc, in_maps, list(range(NCORES)))
    n = feature.shape[0]
    ntiles = pn // P
    h = np.zeros((NCORES * pn, OUT_DIM), dtype=np.float32)
    for c in range(NCORES):
        hc = np.asarray(res.results[c]["h"]).astype(np.float32)
        # h DRAM layout [128 p, ntiles*64] -> rows t*128+p
        hc = hc.reshape(P, ntiles, OUT_DIM).transpose(1, 0, 2).reshape(pn,
                                                                       OUT_DIM)
        h[c * pn:(c + 1) * pn] = hc
    return np.ascontiguousarray(h[:n], dtype=np.float32)


# revision 15
# speedup vs baseline: 2.1144x; 1.0019x over previous
"""Trainium2 Bass kernel for nn_CGATLayer (GNN message passing).

Algorithm (matches reference):
    z = feature @ fc_weight                      # [N, D]
    s = z @ attn[:D];  d = z @ attn[D:]          # per-node scalars
    e[n,j]   = leaky_relu(s[src[n,j]] + d[n])
    alpha[n,j] = sum_k relu(e[n,j] - e[n,k])
    h[n]     = sum_j alpha[n,j] * z[src[n,j]]

Device strategy (8 NeuronCores, SPMD single program):
  im2col formulation: instead of computing z per node and gathering rows
  per edge (DMA-descriptor bound: one descriptor per edge), the host ships
  the feature matrix with columns replicated in per-edge order
  (featdup[:, (t, j, p)] = feature[src[t*128+p, j]].T, j=32 = the dest's
  own feature column).  Each dest tile then needs 33 PE matmuls
  [128x128]x[128x66] against R = [fc | 0.5*fc@a1 | 0.5*fc@a2] to produce
  per-edge [z | s2 | d2] directly in PSUM -- per-edge data delivery rides
  on contiguous full-bandwidth DMA + the idle PE array instead of 200k
  512-byte gather descriptors.

  Using e' = e/2 (positive homogeneity of leaky_relu/relu):
      alpha = sum_k |e'_j - e'_k| + DEG*e'_j - sum_k e'_k
  so the pairwise clamp reduction is one abs-reduce.

  Per tile: 33 matmuls (5 PSUM banks) -> Act evacuates each bank
  transposed/bf16 into zsT [128, 66, 33] (d-major so the j-reduction is
  innermost) -> Act e' = Lrelu(s2 + d2) with free row-sum accumulator ->
  DVE pairwise diff (bf16 out) -> Pool abs-reduce -> DVE alpha, alpha
  broadcast to [128, 64, 32] (4x tensor_copy), product, one fold ->
  Pool reduces [128, 64, 16] -> h slot; h written to DRAM every 8 tiles
  in a partition-major layout so each partition's rows are contiguous.
"""

from contextlib import ExitStack

import numpy as np

import concourse.bass as bass
import concourse.bacc as bacc
import concourse.tile as tile
from concourse import mybir

F32 = mybir.dt.float32
BF16 = mybir.dt.bfloat16
ALU = mybir.AluOpType
AXL = mybir.AxisListType
ACT = mybir.ActivationFunctionType

N, DEG, IN_DIM, OUT_DIM = 50000, 32, 128, 64
NCORES = 8
NEG_SLOPE = 0.01
P = 128
PN = 6272                    # dest rows per core (49 tiles of 128)
NTILES = PN // P
JB = DEG + 1                 # 32 edge blocks + 1 own-feature block (d2)
OC = OUT_DIM + 2             # matmul output cols: z(64) | s2 | d2
HGRP = 8                     # tiles batched per h write


def build_program(pn=PN, in_dim=IN_DIM, out_dim=OUT_DIM, ncores=NCORES):
    ntiles = pn // P
    bank_js = [7, 7, 7, 7, 5]            # j-blocks per PSUM bank (sum=33)
    nh = (ntiles + HGRP - 1) // HGRP

    nc = bacc.Bacc("TRN2", num_devices=ncores)
    fd = nc.declare_dram_parameter("fd", [in_dim, ntiles * JB * P], BF16,
                                   isOutput=False)
    fc = nc.declare_dram_parameter("fc", [in_dim, out_dim], BF16, isOutput=False)
    fcT = nc.declare_dram_parameter("fcT", [out_dim, in_dim], BF16,
                                    isOutput=False)
    attn2 = nc.declare_dram_parameter("attn2", [out_dim, 2], BF16,
                                      isOutput=False)
    h = nc.declare_dram_parameter("h", [P, ntiles * out_dim], F32,
                                  isOutput=True)

    with tile.TileContext(nc) as tc, ExitStack() as ctx:
        const_pool = ctx.enter_context(tc.tile_pool(name="const", bufs=1))
        fd_pool = ctx.enter_context(tc.tile_pool(name="fd", bufs=3))
        ps_pool = ctx.enter_context(tc.tile_pool(name="ps", bufs=1,
                                                 space="PSUM"))
        zs_pool = ctx.enter_context(tc.tile_pool(name="zs", bufs=8))
        sm_pool = ctx.enter_context(tc.tile_pool(name="sm", bufs=7))
        D_pool = ctx.enter_context(tc.tile_pool(name="Dp", bufs=4))
        pr_pool = ctx.enter_context(tc.tile_pool(name="pr", bufs=4))
        h_pool = ctx.enter_context(tc.tile_pool(name="hp", bufs=2))

        # ---- weight prep: R = [fc | 0.5*fc@a1 | 0.5*fc@a2]  [in_dim, 66] bf16
        # (attn2 is pre-scaled by 0.5 on the host for both columns)
        fc_sb = const_pool.tile([in_dim, out_dim], BF16)
        nc.sync.dma_start(fc_sb[:], fc[:])
        fcT_sb = const_pool.tile([out_dim, in_dim], BF16)
        nc.sync.dma_start(fcT_sb[:], fcT[:])
        attn2_sb = const_pool.tile([out_dim, 2], BF16)
        nc.sync.dma_start(attn2_sb[:], attn2[:])
        R_sb = const_pool.tile([in_dim, OC], BF16)
        wsd_ps = ps_pool.tile([in_dim, 2], F32, tag="psW")
        nc.tensor.matmul(out=wsd_ps[:], lhsT=fcT_sb[:], rhs=attn2_sb[:],
                         start=True, stop=True)
        nc.vector.tensor_copy(out=R_sb[:, 0:out_dim], in_=fc_sb[:])
        nc.vector.tensor_copy(out=R_sb[:, out_dim:OC], in_=wsd_ps[:])

        st = {}

        def stage1(t):
            """DMA + matmuls + evac + e' for tile t."""
            fdt = fd_pool.tile([in_dim, JB * P], BF16, tag="fd")
            nc.sync.dma_start(fdt[:], fd[:, t * JB * P:(t + 1) * JB * P])

            # zsT[p, d, j]: d-major per-edge [z | s2 | d2] so the j-axis
            # (reduced later) is innermost
            zsT = zs_pool.tile([P, OC * JB], BF16, tag="zs")
            zsT3 = zsT[:].rearrange("p (d j) -> p d j", j=JB)
            j0 = 0
            for b, nj in enumerate(bank_js):
                ps = ps_pool.tile([P, nj * OC], F32, tag=f"ps{b}")
                for q in range(nj):
                    j = j0 + q
                    nc.tensor.matmul(out=ps[:, q * OC:(q + 1) * OC],
                                     lhsT=fdt[:, j * P:(j + 1) * P],
                                     rhs=R_sb[:], start=True, stop=True)
                # evacuate transposed (d-major) + f32 -> bf16
                ps3 = ps[:].rearrange("p (j d) -> p j d", d=OC)
                nc.scalar.activation(out=zsT3[:, :, j0:j0 + nj],
                                     in_=ps3.transpose([0, 2, 1]),
                                     func=ACT.Copy)
                j0 += nj

            # e' = Lrelu(s2 + d2); sumE = sum_j e'  (free accumulator)
            e = sm_pool.tile([P, DEG], F32, tag="e")
            sumE = sm_pool.tile([P, 1], F32, tag="sumE")
            s2v = zsT[:][:, out_dim * JB:out_dim * JB + DEG]
            d2v = zsT[:][:, (out_dim + 1) * JB + DEG:(out_dim + 1) * JB + DEG + 1]
            nc.scalar.activation(out=e[:], in_=s2v, func=ACT.Lrelu,
                                 bias=d2v, scale=1.0, alpha=NEG_SLOPE,
                                 accum_out=sumE[:])
            st[t] = {"zsT3": zsT3, "e": e, "sumE": sumE}

        def stage2(t):
            """D (Pool) + al0 (DVE), inputs one iteration old."""
            s = st[t]
            D = D_pool.tile([P, DEG * DEG], BF16, tag="D")
            D3 = D[:].rearrange("p (j k) -> p j k", k=DEG)
            e = s["e"]
            nc.gpsimd.tensor_tensor(
                out=D3, in0=e[:].unsqueeze(2).broadcast_to([P, DEG, DEG]),
                in1=e[:].unsqueeze(1).broadcast_to([P, DEG, DEG]),
                op=ALU.subtract)
            al0 = sm_pool.tile([P, DEG], F32, tag="al0")
            nc.vector.tensor_scalar(out=al0[:], in0=e[:], scalar1=float(DEG),
                                    scalar2=s["sumE"][:], op0=ALU.mult,
                                    op1=ALU.subtract)
            s["D3"], s["al0"] = D3, al0

        def stage3(t):
            """A = sum_k |D|  (DVE)."""
            s = st[t]
            A = sm_pool.tile([P, DEG], F32, tag="A")
            nc.vector.tensor_reduce(out=A[:], in_=s["D3"], axis=AXL.X,
                                    op=ALU.add, apply_absolute_value=True)
            s["A"] = A

        def stage4(t):
            """alpha = A + al0  (Pool; bf16 for the weighted sum)."""
            s = st[t]
            alpha = sm_pool.tile([P, DEG], BF16, tag="alpha")
            nc.gpsimd.tensor_tensor(out=alpha[:], in0=s["al0"][:],
                                    in1=s["A"][:], op=ALU.add)
            s["alpha"] = alpha

        def stage5(t):
            """prod + fold 32->16  (DVE); alpha rides as a broadcast view
            (middle-dim stride 0, packed last => 2x)."""
            s = st[t]
            prod = pr_pool.tile([P, out_dim * DEG], BF16, tag="prod")
            prod3 = prod[:].rearrange("p (d j) -> p d j", j=DEG)
            nc.vector.tensor_tensor(
                out=prod3, in0=s["zsT3"][:, 0:out_dim, 0:DEG],
                in1=s["alpha"][:].unsqueeze(1).broadcast_to(
                    [P, out_dim, DEG]),
                op=ALU.mult)
            ph = pr_pool.tile([P, out_dim * (DEG // 2)], BF16, tag="ph")
            ph3 = ph[:].rearrange("p (d j) -> p d j", j=DEG // 2)
            nc.vector.tensor_tensor(out=ph3, in0=prod3[:, :, 0:DEG // 2],
                                    in1=prod3[:, :, DEG // 2:DEG], op=ALU.add)
            s["ph3"] = ph3

        def stage6(t):
            """fold 16->8  (Pool)."""
            s = st[t]
            pq = pr_pool.tile([P, out_dim * (DEG // 4)], BF16, tag="pq")
            pq3 = pq[:].rearrange("p (d j) -> p d j", j=DEG // 4)
            nc.gpsimd.tensor_tensor(out=pq3, in0=s["ph3"][:, :, 0:DEG // 4],
                                    in1=s["ph3"][:, :, DEG // 4:DEG // 2],
                                    op=ALU.add)
            s["pq3"] = pq3

        def stage7(t):
            """reduce 8->1 into the h slot (DVE); h DMA every HGRP tiles."""
            nonlocal hbuf
            s = st.pop(t)
            if t % HGRP == 0:
                hbuf = h_pool.tile([P, HGRP * out_dim], F32, tag="hbuf")
            sl = t % HGRP
            nc.vector.tensor_reduce(
                out=hbuf[:, sl * out_dim:(sl + 1) * out_dim],
                in_=s["pq3"], axis=AXL.X, op=ALU.add)
            if t % HGRP == HGRP - 1 or t == ntiles - 1:
                g0 = (t // HGRP) * HGRP
                w = (t - g0 + 1) * out_dim
                nc.sync.dma_start(out=h[:, g0 * out_dim:g0 * out_dim + w],
                                  in_=hbuf[:, 0:w])

        hbuf = None
        phases = [(stage1, 0), (stage2, 2), (stage3, 4), (stage4, 5),
                  (stage5, 6), (stage6, 7), (stage7, 8)]
        depth = max(k for _, k in phases)
        for i in range(ntiles + depth):
            for phase, k in phases:
                t = i - k
                if 0 <= t < ntiles:
                    phase(t)

    nc.compile()
    return nc


def prep_inputs(feature, src_idx, fc_weight, attn_weight, ncores=NCORES):
    """Host-side sharding/layout prep -> per-core input maps."""
    import ml_dtypes

    bf16 = ml_dtypes.bfloat16
    feature = np.asarray(feature, dtype=np.float32)
    src = np.asarray(src_idx).astype(np.int64)
    fcw = np.asarray(fc_weight, dtype=np.float32)
    aw = np.asarray(attn_weight, dtype=np.float32)
    n, in_dim = feature.shape
    out_dim = fcw.shape[1]
    deg = src.shape[1]
    pn = PN
    ntiles = pn // P

    featT = np.ascontiguousarray(feature.T).astype(bf16)
    fcb = fcw.astype(bf16)
    fcT = np.ascontiguousarray(fcw.T).astype(bf16)
    attn2 = np.ascontiguousarray(
        np.stack([0.5 * aw[:out_dim, 0], 0.5 * aw[out_dim:, 0]],
                 axis=1)).astype(bf16)

    # padded per-core dest rows (cores own [c*pn, (c+1)*pn); rows >= n are
    # dummies whose outputs are discarded)
    src_pad = np.zeros((ncores * pn, deg), dtype=np.int64)
    src_pad[:n] = src
    featT_pad = np.zeros((in_dim, ncores * pn), dtype=bf16)
    featT_pad[:, :n] = featT

    in_maps = []
    for c in range(ncores):
        rows = np.arange(c * pn, (c + 1) * pn)
        cols = src_pad[c * pn:(c + 1) * pn]                 # [pn, deg]
        idx = np.concatenate([cols, rows[:, None]], axis=1)  # [pn, 33]
        # col order within a tile: j-major then p  -> (t, j, p)
        idx = idx.reshape(ntiles, P, JB).transpose(0, 2, 1).reshape(-1)
        fdc = np.ascontiguousarray(featT_pad[:, idx])
        in_maps.append({"fd": fdc, "fc": fcb, "fcT": fcT, "attn2": attn2})
    return in_maps, pn


_prog_cache = {}


def kernel(feature, src_idx, fc_weight, attn_weight):
    from concourse.bass_utils import run_bass_kernel_spmd

    in_maps, pn = prep_inputs(feature, src_idx, fc_weight, attn_weight)
    key = ("v6", feature.shape, pn)
    if key not in _prog_cache:
        _prog_cache[key] = build_program(pn=pn)
    nc = _prog_cache[key]
    res = run_bass_kernel_spmd(nc, in_maps, list(range(NCORES)))
    n = feature.shape[0]
    ntiles = pn // P
    h = np.zeros((NCORES * pn, OUT_DIM), dtype=np.float32)
    for c in range(NCORES):
        hc = np.asarray(res.results[c]["h"]).astype(np.float32)
        # h DRAM layout [128 p, ntiles*64] -> rows t*128+p
        hc = hc.reshape(P, ntiles, OUT_DIM).transpose(1, 0, 2).reshape(pn,
                                                                       OUT_DIM)
        h[c * pn:(c + 1) * pn] = hc
    return np.ascontiguousarray(h[:n], dtype=np.float32)
